# revision 17
# baseline (speedup 1.0000x reference)
"""Trainium2 Bass kernel for CrossTokenMLPAggregator (top-k masked attention aggregation).

Computes, for full inputs
    mlp_hidden   [B=2, T=2048, H=1024] f32
    attn_weights [B=2, Hh=16, T=2048, T=2048] f32
the reference:
    W = attn_weights.mean(axis=1)              # [B, T, T]
    keep top-8 per query row, renormalize kept mass to sum 1
    out = einsum('bts,bsh->bth', W_sparse, mlp_hidden)

Sharding: 8 cores, each owns 512 query rows (core c -> batch c//4,
query rows (c%4)*512 ...). Each core streams its [16, 512, 2048] slice of
attn_weights (the 512 MiB input dominates; the split is exact), sums the
heads sequentially (bit-exact with the reference's mean accumulation
order, so the top-8 selection matches exactly), finds the top-8 with the
DVE max8 instruction, masks with (W >= v8)*W in one scalar_tensor_tensor
op, transposes the masked rows on the TensorEngine and contracts in bf16
(fp32 PSUM accumulate) against the bf16 mlp_hidden slice resident in
SBUF. Renormalization (1/kept fp32 mass) rides the PSUM->SBUF eviction
on the ScalarEngine; outputs are stored bf16 and upcast to f32 on the
host (~0.1% extra rounding on a 2e-2 budget).

Schedule notes (profiled: the stream sustains ~420 GB/s with 2 MiB
DMAs, 1 MiB only ~340, 512 KiB ~280; DVE tensor_tensor f32 runs
~114 G elem/s, so a full-width add is ~2.3 us and 15 of them + mask +
max8 fill ~99% of the 40 us/tile stream window - DVE is the second
roofline by design):
- EVERY stream DMA is a full 2 MiB head pair; the first pair is fused
  into acc with one two-operand add (acc = h0 + h1, identical
  arithmetic order), so there are exactly 8 pair DMAs per tile and no
  sub-2 MiB transfers anywhere in the attn stream. Shaping the last
  pair smaller to start its adds earlier is a wash: the lost DMA rate
  on sub-1 MiB transfers cancels the earlier add start.
- The head-add chain must stay entirely on DVE: offloading the right
  s-half to GpSimd knocks BOTH engines off their SBUF fast path
  (measured: DVE adds 2.3us -> 3.2us, GpSimd 3.8us per half-add, kernel
  +77us), and GpSimd also rejects scalar_tensor_tensor with a
  per-partition scalar ptr (Pool engine check).
- Mid-stream epilogues mask in two s-halves; the TAIL epilogue masks in
  four s-quarters, each feeding its 4-chunk transpose group, so the
  first matmul starts ~0.65us after v8 instead of ~1.3us. PSUM->SBUF
  evictions of transposed chunks alternate Scalar/Vector in the tail
  (DVE is free there) and all go to Scalar mid-stream. Tail matmuls run
  nh-major so the first output half stores while the second computes.
- Per-tile epilogues are software-pipelined one tile late, emitted
  mid-way through the next tile's add chain. Each mid tile's two output
  evictions share one SBUF tile and one 256 KiB store (two 128 KiB
  stores drag the SDMA packet round-robin).
- HAM power management runs the PE on a ~25% full-clock duty budget in
  3.413us quanta: activity triggers a grant of <= 3 quanta (10.24us),
  the grant is RELEASED as soon as the PE idles, and a cooldown of
  ~3x the granted quanta follows at half clock. Pre-triggering with
  throwaway transposes therefore BACKFIRES - the burst's grant is
  released in the dead gap before the tail matmuls and the forced
  cooldown lands exactly on them (measured: grant 193.9us released
  after 3.4us, cooldown covered the first 16 tail matmuls). So the PE
  is kept silent through the last tile's stream and the tail's own
  transposes trigger the grant naturally.
- The mlp load is staged fp32 in chunk pairs (1 MiB DMAs) on the SCALAR
  HWDGE ring and cast to bf16 on the ScalarEngine, keeping the Sync
  ring pure attn; output DMAs also issue from the Scalar ring.
"""

import numpy as np

B, T, H, Hh, K = 2, 2048, 1024, 16, 8
NCORES = 8
QPC = (B * T) // NCORES          # 512 query rows per core
P = 128                          # partitions
TQ_TILES = QPC // P              # 4 tiles of 128 query rows
S_CHUNKS = T // P                # 16 contraction chunks
HF = T // 2
EPS_SUM = np.float32(1e-8) * np.float32(16.0)  # EPS in head-sum domain

_compiled = {}


def _build_nc():
    import concourse.bass as bass
    import concourse.bacc as bacc
    import concourse.mybir as mybir
    import concourse.tile as tile
    from concourse import masks

    f32 = mybir.dt.float32
    bf16 = mybir.dt.bfloat16
    nc = bacc.Bacc(
        "TRN2",
        target_bir_lowering=False,
        debug=False,
        enable_asserts=False,
        num_devices=NCORES,
    )
    attn = nc.dram_tensor("attn", [Hh, QPC, T], f32, kind="ExternalInput").ap()
    mlp = nc.dram_tensor("mlp", [T, H], f32, kind="ExternalInput").ap()
    out = nc.dram_tensor("out", [QPC, H], bf16, kind="ExternalOutput").ap()

    with tile.TileContext(nc) as tc:
        with (
            tc.tile_pool(name="persist", bufs=1) as persist,
            tc.tile_pool(name="heads", bufs=6) as heads,
            tc.tile_pool(name="headlr", bufs=1) as headlr,
            tc.tile_pool(name="acc", bufs=3) as accp,
            tc.tile_pool(name="stage", bufs=2) as stagep,
            tc.tile_pool(name="wm", bufs=1) as wmp,
            tc.tile_pool(name="wmt", bufs=1) as wmtp,
            tc.tile_pool(name="small", bufs=2) as small,
            tc.tile_pool(name="outsb", bufs=2) as outsbp,
            tc.tile_pool(name="tp_psum", bufs=2, space="PSUM") as tp_psum,
            tc.tile_pool(name="mm_psum", bufs=2, space="PSUM") as mm_psum,
        ):
            mlp_sb = persist.tile([P, S_CHUNKS, H], bf16)
            ident_bf = persist.tile([P, P], bf16)
            masks.make_identity(nc, ident_bf[:])
            ident = persist.tile([P, P], f32)
            masks.make_identity(nc, ident[:])

            LEFT = slice(0, HF)
            RIGHT = slice(HF, T)

            def load_mlp_all():
                # fp32 chunk PAIRS (1 MiB DMAs - 512 KiB transfers drag
                # the shared SDMA packet round-robin and dipped the attn
                # stream to ~345 GB/s during staging) via the Scalar
                # HWDGE ring, cast bf16 on ScalarE
                mlp2 = mlp.rearrange(
                    "(a b p) h -> a b p h", a=S_CHUNKS // 2, b=2, p=P
                )
                for c2 in range(S_CHUNKS // 2):
                    st = stagep.tile([P, 2, H], f32, tag="st")
                    nc.scalar.dma_start(
                        out=st, in_=mlp2[c2].transpose([1, 0, 2])
                    )
                    nc.scalar.copy(mlp_sb[:, 2 * c2 : 2 * c2 + 2, :], st)

            def transpose_group(wm, wmt, g, tail=False):
                # transpose one 4-chunk group; in the tail (DVE is
                # otherwise done) evictions alternate Scalar/Vector so
                # they pipeline instead of serializing on one engine;
                # mid-stream they all go to Scalar since DVE runs at
                # ~99% on the add chain
                pt = tp_psum.tile([P, 4 * P], bf16, tag="ptb")
                for j in range(4):
                    c = 4 * g + j
                    nc.tensor.transpose(
                        pt[:, j * P : (j + 1) * P],
                        wm[:, c * P : (c + 1) * P],
                        ident_bf[:],
                    )
                if tail and g % 2 == 1:
                    nc.vector.tensor_copy(
                        out=wmt[:, 4 * g : 4 * g + 4, :], in_=pt[:]
                    )
                else:
                    nc.scalar.copy(wmt[:, 4 * g : 4 * g + 4, :], pt[:])

            def epilogue(acc, mx, q, tail=False):
                # mask with the row's 8th-largest; s-halves mid-stream,
                # s-quarters in the tail so the first transposes (and the
                # matmuls chasing them) start ~0.65us after v8 lands
                v8 = mx[:, K - 1 : K]
                wm = wmp.tile([P, T], bf16, tag="wm")
                wmt = wmtp.tile([P, S_CHUNKS, P], bf16, tag="wmt")
                stot = small.tile([P, 1], f32, tag="stot")
                nparts = 4 if tail else 2
                PW = T // nparts
                ssum = small.tile([P, nparts], f32, tag="ssum4" if tail else "ssum2")
                for p_ in range(nparts):
                    sl = slice(p_ * PW, (p_ + 1) * PW)
                    nc.vector.scalar_tensor_tensor(
                        out=wm[:, sl],
                        in0=acc[:, sl],
                        scalar=v8,
                        in1=acc[:, sl],
                        op0=mybir.AluOpType.is_ge,
                        op1=mybir.AluOpType.mult,
                        accum_out=ssum[:, p_ : p_ + 1],
                    )
                for g in range(4):
                    transpose_group(wm, wmt, g, tail=tail)
                if tail:
                    s2 = small.tile([P, 2], f32, tag="s2")
                    nc.vector.tensor_add(
                        out=s2, in0=ssum[:, 0:2], in1=ssum[:, 2:4]
                    )
                    nc.vector.tensor_add(out=stot, in0=s2[:, 0:1], in1=s2[:, 1:2])
                else:
                    nc.vector.tensor_add(
                        out=stot, in0=ssum[:, 0:1], in1=ssum[:, 1:2]
                    )
                nc.vector.tensor_scalar_max(stot, stot, float(EPS_SUM))
                rcp = small.tile([P, 1], f32, tag="rcp")
                nc.vector.reciprocal(rcp, stot)

                ps0 = mm_psum.tile([P, 512], f32, tag="ps0")
                ps1 = mm_psum.tile([P, 512], f32, tag="ps1")
                ps = [ps0, ps1]
                osb = outsbp.tile([P, H], bf16, tag="osb")

                def evict(nh):
                    nsl = slice(nh * 512, (nh + 1) * 512)
                    nc.scalar.activation(
                        out=osb[:, nsl],
                        in_=ps[nh][:, :],
                        func=mybir.ActivationFunctionType.Copy,
                        scale=rcp[:, :],
                    )

                if tail:
                    # nh-major: first output half evicts + stores while the
                    # second half's matmuls run (separate PSUM tiles per half
                    # so the second half's start isn't fenced on the first
                    # half's eviction read)
                    for nh in range(H // 512):
                        nsl = slice(nh * 512, (nh + 1) * 512)
                        for c in range(S_CHUNKS):
                            nc.tensor.matmul(
                                ps[nh][:, :],
                                lhsT=wmt[:, c, :],
                                rhs=mlp_sb[:, c, nsl],
                                start=(c == 0),
                                stop=(c == S_CHUNKS - 1),
                            )
                        evict(nh)
                        nc.scalar.dma_start(out=out[q, nsl], in_=osb[:, nsl])
                else:
                    for c in range(S_CHUNKS):
                        for nh in range(H // 512):
                            nsl = slice(nh * 512, (nh + 1) * 512)
                            nc.tensor.matmul(
                                ps[nh][:, :],
                                lhsT=wmt[:, c, :],
                                rhs=mlp_sb[:, c, nsl],
                                start=(c == 0),
                                stop=(c == S_CHUNKS - 1),
                            )
                    for nh in range(H // 512):
                        evict(nh)
                    # one 256 KiB store instead of two 128 KiB ones
                    nc.scalar.dma_start(out=out[q, :], in_=osb[:, :])

            pending = []  # deferred (acc, mx, q) epilogues

            def run_pending():
                if pending:
                    epilogue(*pending.pop())

            def accumulate(acc, q, mid_cb=None, upto=Hh):
                # Sequential h order on DVE: bit-exact with the reference
                # mean's accumulation order, so top-8 selection matches.
                # Heads arrive PAIRED in 2 MiB DMAs; the first pair is
                # fused into acc with one two-operand add (identical
                # arithmetic order).
                first = True
                for hp in range(0, upto, 2):
                    ht2 = heads.tile([P, 2, T], f32, tag="ht2")
                    nc.sync.dma_start(
                        out=ht2,
                        in_=attn[hp : hp + 2, q, :].transpose([1, 0, 2]),
                    )
                    if first:
                        nc.vector.tensor_add(
                            out=acc, in0=ht2[:, 0, :], in1=ht2[:, 1, :]
                        )
                        first = False
                    else:
                        for j in range(2):
                            nc.vector.tensor_add(
                                out=acc, in0=acc, in1=ht2[:, j, :]
                            )
                    if mid_cb is not None and hp == 0:
                        # pipelined previous-tile epilogue, emitted right
                        # after the first fused add: its PE burst fires a
                        # HAM grant whose ~27us cooldown then expires
                        # just before the tail matmuls need full clock
                        # (at hp==6 the cooldown gated the tail until
                        # 206us); the 2.9us of mask DVE work is absorbed
                        # by the add chain's per-pair slack
                        mid_cb()

            load_mlp_all()

            for t in range(TQ_TILES - 1):
                q = slice(t * P, (t + 1) * P)
                acc = accp.tile([P, T], f32, tag="acc")
                accumulate(acc, q)
                mx = small.tile([P, K], f32, tag="mx")
                nc.vector.max(out=mx, in_=acc)
                run_pending()
                pending.append((acc, mx, q))

            # Last tile. The final h14/h15 pair arrives as two COLUMN-half
            # 1 MiB DMAs: the byte stream finishes at full rate either
            # way, but the completion semaphore (slowest of 16 SDMA
            # engines + HBM write receipt, ~3us measured) is what gates
            # the final adds - with the split, the left half's sem fires
            # ~3us earlier and its adds + left max8 run while the right
            # half's receipt settles. Adds stay in h order per column.
            t = TQ_TILES - 1
            q = slice(t * P, (t + 1) * P)
            acc = accp.tile([P, T], f32, tag="acc")
            accumulate(acc, q, mid_cb=run_pending, upto=Hh - 2)
            hl = headlr.tile([P, 2, HF], f32, tag="hl")
            nc.sync.dma_start(
                out=hl, in_=attn[Hh - 2 :, q, LEFT].transpose([1, 0, 2])
            )
            hr = headlr.tile([P, 2, HF], f32, tag="hr")
            nc.sync.dma_start(
                out=hr, in_=attn[Hh - 2 :, q, RIGHT].transpose([1, 0, 2])
            )
            mxh = small.tile([P, 2 * K], f32, tag="mxh")
            nc.vector.tensor_add(out=acc[:, LEFT], in0=acc[:, LEFT], in1=hl[:, 0, :])
            nc.vector.tensor_add(out=acc[:, LEFT], in0=acc[:, LEFT], in1=hl[:, 1, :])
            nc.vector.max(out=mxh[:, 0:K], in_=acc[:, LEFT])
            # bridge burst: throwaway transposes dep'd on the left-half
            # final add keep the PE continuously busy from the DVE drain
            # into the real tail transposes - a GAP would release the HAM
            # grant and force a >=10us cooldown over the tail matmuls
            for w in range(10):
                ptw = tp_psum.tile([P, 4 * P], f32, tag="pt")
                nc.tensor.transpose(ptw[:, 0:P], acc[:, 0:P], ident[:])
            nc.vector.tensor_add(out=acc[:, RIGHT], in0=acc[:, RIGHT], in1=hr[:, 0, :])
            nc.vector.tensor_add(out=acc[:, RIGHT], in0=acc[:, RIGHT], in1=hr[:, 1, :])
            nc.vector.max(out=mxh[:, K : 2 * K], in_=acc[:, RIGHT])
            mx = small.tile([P, K], f32, tag="mx")
            nc.vector.max(out=mx, in_=mxh)
            epilogue(acc, mx, q, tail=True)

    nc.compile()
    return nc


def _get_nc():
    if "nc" not in _compiled:
        _compiled["nc"] = _build_nc()
    return _compiled["nc"]


def kernel(mlp_hidden: np.ndarray, attn_weights: np.ndarray) -> np.ndarray:
    from concourse.bass_utils import run_bass_kernel_spmd

    mlp_hidden = np.ascontiguousarray(mlp_hidden, dtype=np.float32)
    attn_weights = np.ascontiguousarray(attn_weights, dtype=np.float32)
    assert mlp_hidden.shape == (B, T, H)
    assert attn_weights.shape == (B, Hh, T, T)

    nc = _get_nc()
    in_maps = []
    for c in range(NCORES):
        b = c // (NCORES // B)
        q0 = (c % (NCORES // B)) * QPC
        in_maps.append(
            {
                "attn": np.ascontiguousarray(attn_weights[b, :, q0 : q0 + QPC, :]),
                "mlp": mlp_hidden[b],
            }
        )
    res = run_bass_kernel_spmd(nc, in_maps, list(range(NCORES)))
    out = np.empty((B, T, H), dtype=np.float32)
    for c in range(NCORES):
        b = c // (NCORES // B)
        q0 = (c % (NCORES // B)) * QPC
        out[b, q0 : q0 + QPC] = res.results[c]["out"].astype(np.float32)
    return out


# revision 19
# speedup vs baseline: 1.0023x; 1.0023x over previous
"""Trainium2 Bass kernel for CrossTokenMLPAggregator (top-k masked attention aggregation).

Computes, for full inputs
    mlp_hidden   [B=2, T=2048, H=1024] f32
    attn_weights [B=2, Hh=16, T=2048, T=2048] f32
the reference:
    W = attn_weights.mean(axis=1)              # [B, T, T]
    keep top-8 per query row, renormalize kept mass to sum 1
    out = einsum('bts,bsh->bth', W_sparse, mlp_hidden)

Sharding: 8 cores, each owns 512 query rows (core c -> batch c//4,
query rows (c%4)*512 ...). Each core streams its [16, 512, 2048] slice of
attn_weights (the 512 MiB input dominates; the split is exact), sums the
heads sequentially (bit-exact with the reference's mean accumulation
order, so the top-8 selection matches exactly), finds the top-8 with the
DVE max8 instruction, masks with (W >= v8)*W in one scalar_tensor_tensor
op, transposes the masked rows on the TensorEngine and contracts in bf16
(fp32 PSUM accumulate) against the bf16 mlp_hidden slice resident in
SBUF. Renormalization (1/kept fp32 mass) rides the PSUM->SBUF eviction
on the ScalarEngine; outputs are stored bf16 and upcast to f32 on the
host (~0.1% extra rounding on a 2e-2 budget).

Schedule notes (profiled: the stream sustains ~420 GB/s with 2 MiB
DMAs, 1 MiB only ~340, 512 KiB ~280; DVE tensor_tensor f32 runs
~114 G elem/s, so a full-width add is ~2.3 us and 15 of them + mask +
max8 fill ~99% of the 40 us/tile stream window - DVE is the second
roofline by design):
- EVERY stream DMA is a full 2 MiB head pair; the first pair is fused
  into acc with one two-operand add (acc = h0 + h1, identical
  arithmetic order), so there are exactly 8 pair DMAs per tile and no
  sub-2 MiB transfers anywhere in the attn stream. Shaping the last
  pair smaller to start its adds earlier is a wash: the lost DMA rate
  on sub-1 MiB transfers cancels the earlier add start.
- The head-add chain must stay entirely on DVE: offloading the right
  s-half to GpSimd knocks BOTH engines off their SBUF fast path
  (measured: DVE adds 2.3us -> 3.2us, GpSimd 3.8us per half-add, kernel
  +77us), and GpSimd also rejects scalar_tensor_tensor with a
  per-partition scalar ptr (Pool engine check).
- Mid-stream epilogues mask in two s-halves; the TAIL epilogue masks in
  four s-quarters, each feeding its 4-chunk transpose group, so the
  first matmul starts ~0.65us after v8 instead of ~1.3us. PSUM->SBUF
  evictions of transposed chunks alternate Scalar/Vector in the tail
  (DVE is free there) and all go to Scalar mid-stream. Tail matmuls run
  nh-major so the first output half stores while the second computes.
- Per-tile epilogues are software-pipelined one tile late, emitted
  mid-way through the next tile's add chain. Each mid tile's two output
  evictions share one SBUF tile and one 256 KiB store (two 128 KiB
  stores drag the SDMA packet round-robin).
- HAM power management runs the PE on a ~25% full-clock duty budget in
  3.413us quanta: activity triggers a grant of <= 3 quanta (10.24us),
  the grant is RELEASED as soon as the PE idles, and a cooldown of
  ~3x the granted quanta follows at half clock. Pre-triggering with
  throwaway transposes therefore BACKFIRES - the burst's grant is
  released in the dead gap before the tail matmuls and the forced
  cooldown lands exactly on them (measured: grant 193.9us released
  after 3.4us, cooldown covered the first 16 tail matmuls). So the PE
  is kept silent through the last tile's stream and the tail's own
  transposes trigger the grant naturally.
- The mlp load is staged fp32 in chunk pairs (1 MiB DMAs) on the SCALAR
  HWDGE ring and cast to bf16 on the ScalarEngine, keeping the Sync
  ring pure attn; output DMAs also issue from the Scalar ring.
"""

import numpy as np

B, T, H, Hh, K = 2, 2048, 1024, 16, 8
NCORES = 8
QPC = (B * T) // NCORES          # 512 query rows per core
P = 128                          # partitions
TQ_TILES = QPC // P              # 4 tiles of 128 query rows
S_CHUNKS = T // P                # 16 contraction chunks
HF = T // 2
EPS_SUM = np.float32(1e-8) * np.float32(16.0)  # EPS in head-sum domain

_compiled = {}


def _build_nc():
    import concourse.bass as bass
    import concourse.bacc as bacc
    import concourse.mybir as mybir
    import concourse.tile as tile
    from concourse import masks

    f32 = mybir.dt.float32
    bf16 = mybir.dt.bfloat16
    nc = bacc.Bacc(
        "TRN2",
        target_bir_lowering=False,
        debug=False,
        enable_asserts=False,
        num_devices=NCORES,
    )
    attn = nc.dram_tensor("attn", [Hh, QPC, T], f32, kind="ExternalInput").ap()
    mlp = nc.dram_tensor("mlp", [T, H], f32, kind="ExternalInput").ap()
    out = nc.dram_tensor("out", [QPC, H], bf16, kind="ExternalOutput").ap()

    with tile.TileContext(nc) as tc:
        with (
            tc.tile_pool(name="persist", bufs=1) as persist,
            tc.tile_pool(name="heads", bufs=3) as heads,
            tc.tile_pool(name="acc", bufs=3) as accp,
            tc.tile_pool(name="stage", bufs=2) as stagep,
            tc.tile_pool(name="wm", bufs=1) as wmp,
            tc.tile_pool(name="wmt", bufs=1) as wmtp,
            tc.tile_pool(name="small", bufs=2) as small,
            tc.tile_pool(name="outsb", bufs=2) as outsbp,
            tc.tile_pool(name="tp_psum", bufs=2, space="PSUM") as tp_psum,
            tc.tile_pool(name="mm_psum", bufs=2, space="PSUM") as mm_psum,
        ):
            mlp_sb = persist.tile([P, S_CHUNKS, H], bf16)
            ident_bf = persist.tile([P, P], bf16)
            masks.make_identity(nc, ident_bf[:])

            LEFT = slice(0, HF)
            RIGHT = slice(HF, T)

            def load_mlp_all():
                # fp32 chunk QUADS (2 MiB DMAs - sub-1 MiB transfers drag
                # the shared SDMA packet round-robin and dipped the attn
                # stream to ~345 GB/s during staging) via the Scalar
                # HWDGE ring, cast bf16 on ScalarE
                mlp4 = mlp.rearrange(
                    "(a b p) h -> a b p h", a=S_CHUNKS // 4, b=4, p=P
                )
                for c4 in range(S_CHUNKS // 4):
                    st = stagep.tile([P, 4, H], f32, tag="st")
                    nc.scalar.dma_start(
                        out=st, in_=mlp4[c4].transpose([1, 0, 2])
                    )
                    nc.scalar.copy(mlp_sb[:, 4 * c4 : 4 * c4 + 4, :], st)

            def transpose_group(wm, wmt, g, tail=False):
                # transpose one 4-chunk group; in the tail (DVE is
                # otherwise done) evictions alternate Scalar/Vector so
                # they pipeline instead of serializing on one engine;
                # mid-stream they all go to Scalar since DVE runs at
                # ~99% on the add chain
                pt = tp_psum.tile([P, 4 * P], bf16, tag="ptb")
                for j in range(4):
                    c = 4 * g + j
                    nc.tensor.transpose(
                        pt[:, j * P : (j + 1) * P],
                        wm[:, c * P : (c + 1) * P],
                        ident_bf[:],
                    )
                if tail and g % 2 == 1:
                    nc.vector.tensor_copy(
                        out=wmt[:, 4 * g : 4 * g + 4, :], in_=pt[:]
                    )
                else:
                    nc.scalar.copy(wmt[:, 4 * g : 4 * g + 4, :], pt[:])

            def epilogue(acc, mx, q, tail=False):
                # mask with the row's 8th-largest; s-halves mid-stream,
                # s-quarters in the tail so the first transposes (and the
                # matmuls chasing them) start ~0.65us after v8 lands
                v8 = mx[:, K - 1 : K]
                wm = wmp.tile([P, T], bf16, tag="wm")
                wmt = wmtp.tile([P, S_CHUNKS, P], bf16, tag="wmt")
                stot = small.tile([P, 1], f32, tag="stot")
                nparts = 4 if tail else 2
                PW = T // nparts
                ssum = small.tile([P, nparts], f32, tag="ssum4" if tail else "ssum2")
                for p_ in range(nparts):
                    sl = slice(p_ * PW, (p_ + 1) * PW)
                    nc.vector.scalar_tensor_tensor(
                        out=wm[:, sl],
                        in0=acc[:, sl],
                        scalar=v8,
                        in1=acc[:, sl],
                        op0=mybir.AluOpType.is_ge,
                        op1=mybir.AluOpType.mult,
                        accum_out=ssum[:, p_ : p_ + 1],
                    )
                for g in range(4):
                    transpose_group(wm, wmt, g, tail=tail)
                if tail:
                    s2 = small.tile([P, 2], f32, tag="s2")
                    nc.vector.tensor_add(
                        out=s2, in0=ssum[:, 0:2], in1=ssum[:, 2:4]
                    )
                    nc.vector.tensor_add(out=stot, in0=s2[:, 0:1], in1=s2[:, 1:2])
                else:
                    nc.vector.tensor_add(
                        out=stot, in0=ssum[:, 0:1], in1=ssum[:, 1:2]
                    )
                nc.vector.tensor_scalar_max(stot, stot, float(EPS_SUM))
                rcp = small.tile([P, 1], f32, tag="rcp")
                nc.vector.reciprocal(rcp, stot)

                ps0 = mm_psum.tile([P, 512], f32, tag="ps0")
                ps1 = mm_psum.tile([P, 512], f32, tag="ps1")
                ps = [ps0, ps1]
                osb = outsbp.tile([P, H], bf16, tag="osb")

                def evict(nh):
                    nsl = slice(nh * 512, (nh + 1) * 512)
                    nc.scalar.activation(
                        out=osb[:, nsl],
                        in_=ps[nh][:, :],
                        func=mybir.ActivationFunctionType.Copy,
                        scale=rcp[:, :],
                    )

                if tail:
                    # nh-major: first output half evicts + stores while the
                    # second half's matmuls run (separate PSUM tiles per half
                    # so the second half's start isn't fenced on the first
                    # half's eviction read)
                    for nh in range(H // 512):
                        nsl = slice(nh * 512, (nh + 1) * 512)
                        for c in range(S_CHUNKS):
                            nc.tensor.matmul(
                                ps[nh][:, :],
                                lhsT=wmt[:, c, :],
                                rhs=mlp_sb[:, c, nsl],
                                start=(c == 0),
                                stop=(c == S_CHUNKS - 1),
                            )
                        evict(nh)
                        nc.scalar.dma_start(out=out[q, nsl], in_=osb[:, nsl])
                else:
                    for c in range(S_CHUNKS):
                        for nh in range(H // 512):
                            nsl = slice(nh * 512, (nh + 1) * 512)
                            nc.tensor.matmul(
                                ps[nh][:, :],
                                lhsT=wmt[:, c, :],
                                rhs=mlp_sb[:, c, nsl],
                                start=(c == 0),
                                stop=(c == S_CHUNKS - 1),
                            )
                    for nh in range(H // 512):
                        evict(nh)
                    # one 256 KiB store instead of two 128 KiB ones
                    nc.scalar.dma_start(out=out[q, :], in_=osb[:, :])

            pending = []  # deferred (acc, mx, q) epilogues

            def run_pending():
                if pending:
                    epilogue(*pending.pop())

            def accumulate(acc, q, pairs=False, mid_cb=None):
                # Sequential h order on DVE: bit-exact with the reference
                # mean's accumulation order, so top-8 selection matches.
                # Heads land in 4-head QUAD buffers; mid tiles fill each
                # with one 4 MiB DMA (best sustained rate), the LAST tile
                # fills each with two 2 MiB pair DMAs so the final
                # transfer stays small enough not to lengthen the
                # post-stream drain. First two heads fuse into acc with
                # one two-operand add (identical arithmetic order).
                for g in range(Hh // 4):
                    ht4 = heads.tile([P, 4, T], f32, tag="ht4")
                    if pairs:
                        nc.sync.dma_start(
                            out=ht4[:, 0:2, :],
                            in_=attn[4 * g : 4 * g + 2, q, :].transpose([1, 0, 2]),
                        )
                        nc.sync.dma_start(
                            out=ht4[:, 2:4, :],
                            in_=attn[4 * g + 2 : 4 * g + 4, q, :].transpose([1, 0, 2]),
                        )
                    else:
                        nc.sync.dma_start(
                            out=ht4,
                            in_=attn[4 * g : 4 * g + 4, q, :].transpose([1, 0, 2]),
                        )
                    js = range(4)
                    if g == 0:
                        nc.vector.tensor_add(
                            out=acc, in0=ht4[:, 0, :], in1=ht4[:, 1, :]
                        )
                        js = range(2, 4)
                    for j in js:
                        nc.vector.tensor_add(
                            out=acc, in0=acc, in1=ht4[:, j, :]
                        )
                    if mid_cb is not None and g == 1:
                        # pipelined previous-tile epilogue, emitted
                        # mid-chain so its DVE ops don't delay the tail
                        # drain
                        mid_cb()

            load_mlp_all()

            for t in range(TQ_TILES):
                q = slice(t * P, (t + 1) * P)
                acc = accp.tile([P, T], f32, tag="acc")
                last = t == TQ_TILES - 1
                accumulate(acc, q, pairs=last, mid_cb=run_pending if last else None)
                mx = small.tile([P, K], f32, tag="mx")
                nc.vector.max(out=mx, in_=acc)
                if last:
                    epilogue(acc, mx, q, tail=True)
                else:
                    run_pending()
                    pending.append((acc, mx, q))

    nc.compile()
    return nc


def _get_nc():
    if "nc" not in _compiled:
        _compiled["nc"] = _build_nc()
    return _compiled["nc"]


def kernel(mlp_hidden: np.ndarray, attn_weights: np.ndarray) -> np.ndarray:
    from concourse.bass_utils import run_bass_kernel_spmd

    mlp_hidden = np.ascontiguousarray(mlp_hidden, dtype=np.float32)
    attn_weights = np.ascontiguousarray(attn_weights, dtype=np.float32)
    assert mlp_hidden.shape == (B, T, H)
    assert attn_weights.shape == (B, Hh, T, T)

    nc = _get_nc()
    in_maps = []
    for c in range(NCORES):
        b = c // (NCORES // B)
        q0 = (c % (NCORES // B)) * QPC
        in_maps.append(
            {
                "attn": np.ascontiguousarray(attn_weights[b, :, q0 : q0 + QPC, :]),
                "mlp": mlp_hidden[b],
            }
        )
    res = run_bass_kernel_spmd(nc, in_maps, list(range(NCORES)))
    out = np.empty((B, T, H), dtype=np.float32)
    for c in range(NCORES):
        b = c // (NCORES // B)
        q0 = (c % (NCORES // B)) * QPC
        out[b, q0 : q0 + QPC] = res.results[c]["out"].astype(np.float32)
    return out
